# revision 34
# baseline (speedup 1.0000x reference)
"""Multi-head attention (B=2, S=2048, E=1024, H=16, causal) on 8 TRN2 cores.

Sharding: core c -> batch b = c//4, head group g = c%4 (4 heads each).
Each core computes QKV projection for its heads, causal flash-style
attention (no-max softmax, denominator via ones-column appended to V),
and a partial output projection against a 256-row slice of W_proj.
Host sums the 4 partial projections per batch (the "all-reduce") and
stacks the 2 batches.

All matmul operands are float32r (TF32-like single-pass PE matmul, fp32
accumulation in PSUM). Activation layouts are chosen so no on-device
transposes are needed: the host passes x[b].T per core.
"""
import sys

sys.path.insert(0, "/opt/trn_rl_repo")

import ml_dtypes
import numpy as np

import concourse.bacc as bacc
import concourse.mybir as mybir
from concourse import tile
from concourse.bass_utils import run_bass_kernel_spmd

B, S, E, H, D = 2, 2048, 1024, 16, 64
SCALE = D ** -0.5
N_CORES = 8
HL = 4            # heads per core
GC = 256          # channel columns per core (HL * D)
F32 = mybir.dt.float32
F32R = mybir.dt.float32r
BF16 = mybir.dt.bfloat16

_CACHED_NC = None


def _build():
    nc = bacc.Bacc("TRN2", target_bir_lowering=False, debug=False,
                   num_devices=N_CORES)

    xT = nc.dram_tensor("xT", [E, S], BF16, kind="ExternalInput")
    w = nc.dram_tensor("w", [E, 3 * GC], BF16, kind="ExternalInput")
    wp = nc.dram_tensor("wp", [GC, E], BF16, kind="ExternalInput")
    mask = nc.dram_tensor("mask", [128, 128], BF16, kind="ExternalInput")
    y = nc.dram_tensor("y", [S, E], F32, kind="ExternalOutput")

    ET = E // 128     # 8 e-tiles
    NS = S // 512     # 4 s-chunks of 512
    NT = S // 128     # 16 s-tiles of 128

    with tile.TileContext(nc) as tc:
        with (
            tc.tile_pool(name="const", bufs=1) as cst,
            tc.tile_pool(name="acts", bufs=1) as acts,
            tc.tile_pool(name="expp", bufs=5) as expp,
            tc.tile_pool(name="small", bufs=2) as small,
            tc.tile_pool(name="ysb", bufs=3) as ysbp,
            tc.tile_pool(name="psS", bufs=2, space="PSUM") as psS,
            tc.tile_pool(name="psO", bufs=4, space="PSUM") as psO,
        ):
            # ---- constant loads -------------------------------------------
            xt = cst.tile([128, ET, S], BF16)          # x[b]^T  (e on partitions)
            wt = cst.tile([128, ET, 3 * GC], BF16)     # W_qkv slice (e on partitions)
            wpt = cst.tile([128, 2, E], BF16)          # W_proj slice (c on partitions)
            mk = cst.tile([128, 128], BF16)            # tril(128) causal mask

            # weights first, then x chunk 0, so QKV of chunk 0 starts ASAP;
            # remaining x chunks stream in behind it
            wr = w[:].rearrange("(t p) j -> p t j", p=128)
            xTr = xT[:].rearrange("(t p) s -> p t s", p=128)
            # first-wave chunks go out on three engines' DMA queues in
            # parallel (single-queue input streaming tops out well below the
            # HBM limit), later chunks stream behind on the sync queue
            nc.sync.dma_start(wt[:, 0:4, 0:512], wr[:, 0:4, 0:512])
            nc.scalar.dma_start(xt[:, 0:4, 0:512], xTr[:, 0:4, 0:512])
            nc.gpsimd.dma_start(wt[:, 4:8, 0:512], wr[:, 4:8, 0:512])
            nc.sync.dma_start(xt[:, 4:8, 0:512], xTr[:, 4:8, 0:512])
            nc.scalar.dma_start(wt[:, :, 512:768], wr[:, :, 512:768])
            nc.gpsimd.dma_start(mk[:], mask[:])
            nc.scalar.dma_start(wpt[:], wp[:].rearrange("(t p) e -> p t e", p=128))
            for sc in range(1, NS):
                nc.sync.dma_start(
                    xt[:, :, 512 * sc:512 * (sc + 1)], xTr[:, :, 512 * sc:512 * (sc + 1)]
                )

            # ---- activation buffers ---------------------------------------
            # qt/kt: [pair, j(128 part: head 2p on 0-63, head 2p+1 on 64-127), s]
            qt = acts.tile([128, 2, S], BF16)
            kt = acts.tile([128, 2, S], BF16)
            # v_aug: per s-tile, per head 128 cols (64 data + 64 ones columns;
            # the ones columns make the attnV matmul replicate the softmax
            # denominator across output partitions 64-127, so normalization
            # needs no partition broadcast)
            vt = acts.tile([128, NT, HL * 128], BF16)
            # attention output^T, proj lhsT layout: c on partitions
            ot = acts.tile([128, 2, S], BF16)

            # dedicated dummy-source tile: no dependence on input DMAs or the
            # big vt memset, so warm-up matmuls start right away
            dum = cst.tile([128, 1024], BF16)
            nc.vector.memset(dum[:], 0.001)
            warm = cst.tile([128, 16], F32)
            nc.scalar.activation(warm[:], dum[:, 0:16],
                                 mybir.ActivationFunctionType.Exp)
            # dummy matmuls during the DMA head keep the PE HAM-warm so QKV
            # starts at full clock
            for wi in range(12):
                pw = psS.tile([128, 1024], F32, tag="ps", name="pw")
                nc.tensor.matmul(pw[:, 0:512], dum[:, 0:128], dum[:, 512:1024],
                                 start=True, stop=True)
            # only the ones-columns of v_aug need initializing; v_chunk fills
            # the data columns
            nc.vector.memset(
                vt[:].rearrange("p t (h m) -> p t h m", h=HL)[:, :, :, 64:128], 1.0)

            # ---- QKV / proj work as generators: one matmul per yield ------
            # These feed a FIFO of filler work that the attention loop pumps
            # between its own matmuls, keeping the in-order PE queue dense
            # while exps run on the ACT engine.
            def gen_qk(sc, jt):
                s0 = 512 * sc
                for which, dest in ((0, qt), (1, kt)):
                    ps = psO.tile([128, 512], F32, tag="po", name="psqk")
                    for et in range(ET):
                        nc.tensor.matmul(
                            ps[:, 0:512],
                            wt[:, et, 256 * which + 128 * jt:256 * which + 128 * (jt + 1)],
                            xt[:, et, s0:s0 + 512],
                            start=(et == 0),
                            stop=(et == ET - 1),
                            skip_group_check=True,
                        )
                        yield
                    nc.vector.tensor_copy(dest[:, jt, s0:s0 + 512], ps[:, 0:512])

            def gen_v(sc):
                for st4 in range(4):
                    st = 4 * sc + st4
                    ps = psO.tile([128, 512], F32, tag="po", name="psv")
                    for et in range(ET):
                        nc.tensor.matmul(
                            ps[:, 0:256],
                            xt[:, et, 128 * st:128 * (st + 1)],
                            wt[:, et, 512:768],
                            start=(et == 0),
                            stop=(et == ET - 1),
                            skip_group_check=True,
                        )
                        yield
                    nc.vector.tensor_copy(
                        vt[:, st].rearrange("p (h m) -> p h m", h=HL)[:, :, 0:64],
                        ps[:, 0:256].rearrange("p (h m) -> p h m", h=HL),
                    )

            def gen_proj(jq, lo=0, hi=8, alternate=False):
                for i in range(lo, hi):
                    st4, nk = divmod(i, 2)
                    st = 4 * jq + st4
                    py = psO.tile([128, 512], F32, tag="po", name="py")
                    for ct in range(2):
                        nc.tensor.matmul(
                            py[:],
                            ot[:, ct, 128 * st:128 * (st + 1)],
                            wpt[:, ct, 512 * nk:512 * (nk + 1)],
                            start=(ct == 0),
                            stop=(ct == 1),
                            skip_group_check=True,
                        )
                        yield
                    ys = ysbp.tile([128, 512], F32)
                    if alternate and i % 2:
                        nc.scalar.copy(ys[:], py[:])
                    else:
                        nc.vector.tensor_copy(ys[:], py[:])
                    nc.sync.dma_start(
                        y[128 * st:128 * (st + 1), 512 * nk:512 * (nk + 1)], ys[:]
                    )

            from collections import deque
            fq = deque()

            def pump(n):
                done = 0
                while fq and done < n:
                    try:
                        next(fq[0])
                        done += 1
                    except StopIteration:
                        fq.popleft()

            def drain(target):
                """run generators from the FIFO head through `target` fully
                (FIFO order preserves inter-generator dependencies)"""
                if target not in fq:
                    return
                while fq:
                    g = fq.popleft()
                    for _ in g:
                        pass
                    if g is target:
                        return

            def run(g):
                for _ in g:
                    pass
            # normalize: out^T[d, s] = o[d, s] * (1 / o[64+d, s]); the ones
            # columns of v_aug put the denominator on partitions 64-127, so
            # this is a partition-shifted reciprocal + multiply, no broadcast.
            # Engines can shift partitions between in and out APs, so head
            # ab=1 writes ot partitions 64-127 directly.
            def attn_norm(pr, jq, o_ab):
                s0 = 512 * jq
                for ab in range(2):
                    o = o_ab[ab]
                    # 1/d = exp(-ln d) on the ACT engine (idle around norms,
                    # and direct Reciprocal is gated off); only the final
                    # multiply rides the busy DVE queue
                    lg = small.tile([128, 512], F32, tag="lg", name="lg")
                    nc.scalar.activation(lg[0:64, :], o[64:128, :],
                                         mybir.ActivationFunctionType.Ln)
                    rinv = small.tile([128, 512], F32, tag="rinv", name="rinv")
                    nc.scalar.activation(rinv[0:64, :], lg[0:64, :],
                                         mybir.ActivationFunctionType.Exp,
                                         scale=-1.0)
                    nc.vector.tensor_mul(ot[64 * ab:64 * ab + 64, pr, s0:s0 + 512],
                                         o[0:64, :], rinv[0:64, :])

            # ---- output projection for one s-chunk ------------------------
            def proj(jq, lo=0, hi=8, alternate=False):
                for i in range(lo, hi):
                    st4, nk = divmod(i, 2)
                    st = 4 * jq + st4
                    py = psO.tile([128, 512], F32, tag="po", name="py")
                    for ct in range(2):
                        nc.tensor.matmul(
                            py[:],
                            ot[:, ct, 128 * st:128 * (st + 1)],
                            wpt[:, ct, 512 * nk:512 * (nk + 1)],
                            start=(ct == 0),
                            stop=(ct == 1),
                        )
                    ys = ysbp.tile([128, 512], F32)
                    if alternate and i % 2:
                        nc.scalar.copy(ys[:], py[:])
                    else:
                        nc.vector.tensor_copy(ys[:], py[:])
                    nc.sync.dma_start(
                        y[128 * st:128 * (st + 1), 512 * nk:512 * (nk + 1)], ys[:]
                    )

            # Software-pipelined attention: scores for ik are issued BEFORE
            # the attnV of ik-1 so the in-order PE queue never parks behind
            # an exp; filler matmuls (QKV/proj) are pumped between them.
            def attn_full(pr, jq, drains, pump_n=2, end=()):
                nik = 4 * jq + 4
                s0 = 512 * jq
                o_ab = [psO.tile([128, 512], F32, tag="po", name="o_ab")
                        for _ in range(2)]
                dr = dict(drains)

                def emit_sc(ik):
                    t = ik - 4 * jq
                    c0 = 128 * t if t > 0 else 0   # exact-causal column trim
                    ps = psS.tile([128, 1024], F32)
                    for ab in range(2):
                        p0 = 64 * ab
                        nc.tensor.matmul(
                            ps[:, 512 * ab + c0:512 * (ab + 1)],
                            kt[p0:p0 + 64, pr, 128 * ik:128 * (ik + 1)],
                            qt[p0:p0 + 64, pr, s0 + c0:s0 + 512],
                            start=True,
                            stop=True,
                            tile_position=(p0, 0),
                            skip_group_check=True,
                        )
                    e = expp.tile([128, 1024], BF16, tag="exps", name="exps")
                    e3 = e[:].rearrange("p (h n) -> p h n", h=2)[:, :, c0:512]
                    ps3 = ps[:].rearrange("p (h n) -> p h n", h=2)[:, :, c0:512]
                    nc.scalar.activation(e3, ps3, mybir.ActivationFunctionType.Exp,
                                         scale=float(SCALE))
                    if t >= 0:
                        # only the first 128 cols of the trimmed range are
                        # partially masked; the rest is fully unmasked
                        for ab in range(2):
                            nc.vector.tensor_mul(
                                e[:, 512 * ab + c0:512 * ab + c0 + 128],
                                e[:, 512 * ab + c0:512 * ab + c0 + 128],
                                mk[:],
                            )
                    return e, c0

                def emit_av(ik, e, c0):
                    for ab in range(2):
                        h = 2 * pr + ab
                        nc.tensor.matmul(
                            o_ab[ab][:, c0:512],
                            vt[:, ik, 128 * h:128 * (h + 1)],
                            e[:, 512 * ab + c0:512 * (ab + 1)],
                            start=(ik == 0),
                            stop=(ik == nik - 1),
                            skip_group_check=True,
                        )

                pend = None
                for ik in range(nik):
                    for g in dr.get(ik, ()):
                        drain(g)
                    e, c0 = emit_sc(ik)
                    if pend is not None:
                        pump(pump_n)
                        emit_av(*pend)
                    pend = (ik, e, c0)
                pump(pump_n)
                emit_av(*pend)
                # drain the next phase's prerequisites BEFORE the norm: their
                # PE work overlaps the last exp, and their qt/kt casts queue
                # on DVE ahead of the norm ops the next phase doesn't need
                for g in end:
                    drain(g)
                attn_norm(pr, jq, o_ab)

            g_qk01 = gen_qk(0, 1)
            g_qk10 = gen_qk(1, 0)
            g_qk11 = gen_qk(1, 1)
            g_qk20 = gen_qk(2, 0)
            g_qk21 = gen_qk(2, 1)
            g_qk30 = gen_qk(3, 0)
            g_qk31 = gen_qk(3, 1)
            g_v1, g_v2, g_v3 = gen_v(1), gen_v(2), gen_v(3)
            g_p0, g_p1, g_p2 = gen_proj(0), gen_proj(1), gen_proj(2)
            fq.extend([g_qk01, g_qk10, g_v1, g_qk11, g_p0, g_qk20, g_v2,
                       g_qk21, g_p1, g_qk30, g_v3, g_qk31, g_p2])

            run(gen_qk(0, 0))
            run(gen_v(0))
            attn_full(0, 0, {}, end=[g_qk01])
            attn_full(1, 0, {}, end=[g_qk10])
            attn_full(0, 1, {4: [g_v1]}, end=[g_qk11])
            attn_full(1, 1, {}, end=[g_qk20])
            attn_full(0, 2, {8: [g_v2]}, end=[g_qk21])
            attn_full(1, 2, {}, end=[g_qk30])
            attn_full(0, 3, {12: [g_v3]}, end=[g_qk31])
            attn_full(1, 3, {}, pump_n=1)
            drain(g_p2)
            run(gen_proj(3, alternate=True))

    nc.compile()
    return nc


def _get_nc():
    global _CACHED_NC
    if _CACHED_NC is None:
        _CACHED_NC = _build()
    return _CACHED_NC


def _diag_masks() -> np.ndarray:
    return np.ascontiguousarray(np.tril(np.ones((128, 128), dtype=np.float32)).T)


def _in_maps(x, W_qkv, W_proj):
    masks = _diag_masks().astype(ml_dtypes.bfloat16)
    maps = []
    for c in range(N_CORES):
        b, g = divmod(c, 4)
        xT = np.ascontiguousarray(x[b].T.astype(ml_dtypes.bfloat16))
        wq = W_qkv[:, GC * g:GC * (g + 1)]
        wk = W_qkv[:, E + GC * g:E + GC * (g + 1)]
        wv = W_qkv[:, 2 * E + GC * g:2 * E + GC * (g + 1)]
        w = np.ascontiguousarray(
            np.concatenate([wq, wk, wv], axis=1).astype(ml_dtypes.bfloat16))
        wp = np.ascontiguousarray(
            W_proj[GC * g:GC * (g + 1), :].astype(ml_dtypes.bfloat16))
        maps.append({"xT": xT, "w": w, "wp": wp, "mask": masks})
    return maps


def _run(x, W_qkv, W_proj, trace=False, **spmd_kwargs):
    nc = _get_nc()
    res = run_bass_kernel_spmd(nc, _in_maps(x, W_qkv, W_proj),
                               list(range(N_CORES)), trace=trace, **spmd_kwargs)
    out = np.zeros((B, S, E), dtype=np.float32)
    for c in range(N_CORES):
        out[c // 4] += res.results[c]["y"]
    return out, res


def kernel(x, attention_mask, W_qkv, W_proj):
    x = np.asarray(x, dtype=np.float32)
    W_qkv = np.asarray(W_qkv, dtype=np.float32)
    W_proj = np.asarray(W_proj, dtype=np.float32)
    out, _ = _run(x, W_qkv, W_proj, trace=False)
    return out



# revision 36
# speedup vs baseline: 1.1651x; 1.1651x over previous
"""Multi-head attention (B=2, S=2048, E=1024, H=16, causal) on 8 TRN2 cores.

Sharding: core c -> batch b = c//4, head group g = c%4 (4 heads each).
Each core computes QKV projection for its heads, causal flash-style
attention (no-max softmax, denominator via ones-column appended to V),
and a partial output projection against a 256-row slice of W_proj.
Host sums the 4 partial projections per batch (the "all-reduce") and
stacks the 2 batches.

All matmul operands are float32r (TF32-like single-pass PE matmul, fp32
accumulation in PSUM). Activation layouts are chosen so no on-device
transposes are needed: the host passes x[b].T per core.
"""
import sys

sys.path.insert(0, "/opt/trn_rl_repo")

import ml_dtypes
import numpy as np

import concourse.bacc as bacc
import concourse.mybir as mybir
from concourse import tile
from concourse.bass_utils import run_bass_kernel_spmd

B, S, E, H, D = 2, 2048, 1024, 16, 64
SCALE = D ** -0.5
N_CORES = 8
HL = 4            # heads per core
GC = 256          # channel columns per core (HL * D)
F32 = mybir.dt.float32
F32R = mybir.dt.float32r
BF16 = mybir.dt.bfloat16

_CACHED_NC = None


def _build():
    nc = bacc.Bacc("TRN2", target_bir_lowering=False, debug=False,
                   num_devices=N_CORES)

    xT = nc.dram_tensor("xT", [E, S], BF16, kind="ExternalInput")
    w = nc.dram_tensor("w", [E, 3 * GC], BF16, kind="ExternalInput")
    wp = nc.dram_tensor("wp", [GC, E], BF16, kind="ExternalInput")
    mask = nc.dram_tensor("mask", [128, 128], BF16, kind="ExternalInput")
    y = nc.dram_tensor("y", [S, E], F32, kind="ExternalOutput")

    ET = E // 128     # 8 e-tiles
    NS = S // 512     # 4 s-chunks of 512
    NT = S // 128     # 16 s-tiles of 128

    with tile.TileContext(nc) as tc:
        with (
            tc.tile_pool(name="const", bufs=1) as cst,
            tc.tile_pool(name="acts", bufs=1) as acts,
            tc.tile_pool(name="expp", bufs=5) as expp,
            tc.tile_pool(name="small", bufs=2) as small,
            tc.tile_pool(name="ysb", bufs=3) as ysbp,
            tc.tile_pool(name="psS", bufs=2, space="PSUM") as psS,
            tc.tile_pool(name="psO", bufs=4, space="PSUM") as psO,
        ):
            # ---- constant loads -------------------------------------------
            xt = cst.tile([128, ET, S], BF16)          # x[b]^T  (e on partitions)
            wt = cst.tile([128, ET, 3 * GC], BF16)     # W_qkv slice (e on partitions)
            wpt = cst.tile([128, 2, E], BF16)          # W_proj slice (c on partitions)
            mk = cst.tile([128, 128], BF16)            # tril(128) causal mask

            # weights first, then x chunk 0, so QKV of chunk 0 starts ASAP;
            # remaining x chunks stream in behind it
            wr = w[:].rearrange("(t p) j -> p t j", p=128)
            xTr = xT[:].rearrange("(t p) s -> p t s", p=128)
            nc.sync.dma_start(wt[:, 0:4, 0:512], wr[:, 0:4, 0:512])
            nc.sync.dma_start(xt[:, 0:4, 0:512], xTr[:, 0:4, 0:512])
            nc.sync.dma_start(wt[:, 4:8, 0:512], wr[:, 4:8, 0:512])
            nc.sync.dma_start(xt[:, 4:8, 0:512], xTr[:, 4:8, 0:512])
            nc.sync.dma_start(wt[:, :, 512:768], wr[:, :, 512:768])
            nc.sync.dma_start(mk[:], mask[:])
            nc.sync.dma_start(wpt[:], wp[:].rearrange("(t p) e -> p t e", p=128))
            for sc in range(1, NS):
                nc.sync.dma_start(
                    xt[:, :, 512 * sc:512 * (sc + 1)], xTr[:, :, 512 * sc:512 * (sc + 1)]
                )

            # ---- activation buffers ---------------------------------------
            # qt/kt: [pair, j(128 part: head 2p on 0-63, head 2p+1 on 64-127), s]
            qt = acts.tile([128, 2, S], BF16)
            kt = acts.tile([128, 2, S], BF16)
            # v_aug: per s-tile, per head 128 cols (64 data + 64 ones columns;
            # the ones columns make the attnV matmul replicate the softmax
            # denominator across output partitions 64-127, so normalization
            # needs no partition broadcast)
            vt = acts.tile([128, NT, HL * 128], BF16)
            # attention output^T, proj lhsT layout: c on partitions
            ot = acts.tile([128, 2, S], BF16)

            # dedicated dummy-source tile: no dependence on input DMAs or the
            # big vt memset, so warm-up matmuls start right away
            dum = cst.tile([128, 1024], BF16)
            nc.vector.memset(dum[:], 0.001)
            warm = cst.tile([128, 16], F32)
            nc.scalar.activation(warm[:], dum[:, 0:16],
                                 mybir.ActivationFunctionType.Exp)
            # dummy matmuls during the DMA head keep the PE HAM-warm so QKV
            # starts at full clock
            for wi in range(12):
                pw = psS.tile([128, 1024], F32, tag="ps", name="pw")
                nc.tensor.matmul(pw[:, 0:512], dum[:, 0:128], dum[:, 512:1024],
                                 start=True, stop=True)
            # only the ones-columns of v_aug need initializing; v_chunk fills
            # the data columns
            nc.vector.memset(
                vt[:].rearrange("p t (h m) -> p t h m", h=HL)[:, :, :, 64:128], 1.0)

            # ---- QKV / proj work as generators: one matmul per yield ------
            # These feed a FIFO of filler work that the attention loop pumps
            # between its own matmuls, keeping the in-order PE queue dense
            # while exps run on the ACT engine.
            def gen_qk(sc, jt):
                s0 = 512 * sc
                for which, dest in ((0, qt), (1, kt)):
                    ps = psO.tile([128, 512], F32, tag="po", name="psqk")
                    for et in range(ET):
                        nc.tensor.matmul(
                            ps[:, 0:512],
                            wt[:, et, 256 * which + 128 * jt:256 * which + 128 * (jt + 1)],
                            xt[:, et, s0:s0 + 512],
                            start=(et == 0),
                            stop=(et == ET - 1),
                            skip_group_check=True,
                        )
                        yield
                    nc.vector.tensor_copy(dest[:, jt, s0:s0 + 512], ps[:, 0:512])

            def gen_v(sc):
                for st4 in range(4):
                    st = 4 * sc + st4
                    ps = psO.tile([128, 512], F32, tag="po", name="psv")
                    for et in range(ET):
                        nc.tensor.matmul(
                            ps[:, 0:256],
                            xt[:, et, 128 * st:128 * (st + 1)],
                            wt[:, et, 512:768],
                            start=(et == 0),
                            stop=(et == ET - 1),
                            skip_group_check=True,
                        )
                        yield
                    nc.vector.tensor_copy(
                        vt[:, st].rearrange("p (h m) -> p h m", h=HL)[:, :, 0:64],
                        ps[:, 0:256].rearrange("p (h m) -> p h m", h=HL),
                    )

            def gen_proj(jq, lo=0, hi=8, alternate=False):
                for i in range(lo, hi):
                    st4, nk = divmod(i, 2)
                    st = 4 * jq + st4
                    py = psO.tile([128, 512], F32, tag="po", name="py")
                    for ct in range(2):
                        nc.tensor.matmul(
                            py[:],
                            ot[:, ct, 128 * st:128 * (st + 1)],
                            wpt[:, ct, 512 * nk:512 * (nk + 1)],
                            start=(ct == 0),
                            stop=(ct == 1),
                            skip_group_check=True,
                        )
                        yield
                    ys = ysbp.tile([128, 512], F32)
                    if alternate and i % 2:
                        nc.scalar.copy(ys[:], py[:])
                    else:
                        nc.vector.tensor_copy(ys[:], py[:])
                    nc.sync.dma_start(
                        y[128 * st:128 * (st + 1), 512 * nk:512 * (nk + 1)], ys[:]
                    )

            from collections import deque
            fq = deque()

            def pump(n):
                done = 0
                while fq and done < n:
                    try:
                        next(fq[0])
                        done += 1
                    except StopIteration:
                        fq.popleft()

            def drain(target):
                """run generators from the FIFO head through `target` fully
                (FIFO order preserves inter-generator dependencies)"""
                if target not in fq:
                    return
                while fq:
                    g = fq.popleft()
                    for _ in g:
                        pass
                    if g is target:
                        return

            def run(g):
                for _ in g:
                    pass
            # normalize: out^T[d, s] = o[d, s] * (1 / o[64+d, s]); the ones
            # columns of v_aug put the denominator on partitions 64-127, so
            # this is a partition-shifted reciprocal + multiply, no broadcast.
            # Engines can shift partitions between in and out APs, so head
            # ab=1 writes ot partitions 64-127 directly.
            def attn_norm(pr, jq, o_ab):
                s0 = 512 * jq
                for ab in range(2):
                    o = o_ab[ab]
                    # denominator PSUM->SBUF bounce on ACT (plain copy: no
                    # activation-table reload, unlike Ln/Reciprocal), recip
                    # and multiply on DVE
                    rs = small.tile([128, 512], F32, tag="rs", name="rs")
                    nc.scalar.copy(rs[0:64, :], o[64:128, :])
                    rinv = small.tile([128, 512], F32, tag="rinv", name="rinv")
                    nc.vector.reciprocal_approx_fast(rinv[0:64, :], rs[0:64, :])
                    nc.vector.tensor_mul(ot[64 * ab:64 * ab + 64, pr, s0:s0 + 512],
                                         o[0:64, :], rinv[0:64, :])

            # ---- output projection for one s-chunk ------------------------
            def proj(jq, lo=0, hi=8, alternate=False):
                for i in range(lo, hi):
                    st4, nk = divmod(i, 2)
                    st = 4 * jq + st4
                    py = psO.tile([128, 512], F32, tag="po", name="py")
                    for ct in range(2):
                        nc.tensor.matmul(
                            py[:],
                            ot[:, ct, 128 * st:128 * (st + 1)],
                            wpt[:, ct, 512 * nk:512 * (nk + 1)],
                            start=(ct == 0),
                            stop=(ct == 1),
                        )
                    ys = ysbp.tile([128, 512], F32)
                    if alternate and i % 2:
                        nc.scalar.copy(ys[:], py[:])
                    else:
                        nc.vector.tensor_copy(ys[:], py[:])
                    nc.sync.dma_start(
                        y[128 * st:128 * (st + 1), 512 * nk:512 * (nk + 1)], ys[:]
                    )

            # Software-pipelined attention: scores for ik are issued BEFORE
            # the attnV of ik-1 so the in-order PE queue never parks behind
            # an exp; filler matmuls (QKV/proj) are pumped between them.
            def attn_full(pr, jq, drains, pump_n=2, end=()):
                nik = 4 * jq + 4
                s0 = 512 * jq
                o_ab = [psO.tile([128, 512], F32, tag="po", name="o_ab")
                        for _ in range(2)]
                dr = dict(drains)

                def emit_sc(ik):
                    t = ik - 4 * jq
                    c0 = 128 * t if t > 0 else 0   # exact-causal column trim
                    ps = psS.tile([128, 1024], F32)
                    for ab in range(2):
                        p0 = 64 * ab
                        nc.tensor.matmul(
                            ps[:, 512 * ab + c0:512 * (ab + 1)],
                            kt[p0:p0 + 64, pr, 128 * ik:128 * (ik + 1)],
                            qt[p0:p0 + 64, pr, s0 + c0:s0 + 512],
                            start=True,
                            stop=True,
                            tile_position=(p0, 0),
                            skip_group_check=True,
                        )
                    e = expp.tile([128, 1024], BF16, tag="exps", name="exps")
                    e3 = e[:].rearrange("p (h n) -> p h n", h=2)[:, :, c0:512]
                    ps3 = ps[:].rearrange("p (h n) -> p h n", h=2)[:, :, c0:512]
                    nc.scalar.activation(e3, ps3, mybir.ActivationFunctionType.Exp,
                                         scale=float(SCALE))
                    if t >= 0:
                        # only the first 128 cols of the trimmed range are
                        # partially masked; the rest is fully unmasked
                        for ab in range(2):
                            nc.vector.tensor_mul(
                                e[:, 512 * ab + c0:512 * ab + c0 + 128],
                                e[:, 512 * ab + c0:512 * ab + c0 + 128],
                                mk[:],
                            )
                    return e, c0

                def emit_av(ik, e, c0):
                    for ab in range(2):
                        h = 2 * pr + ab
                        nc.tensor.matmul(
                            o_ab[ab][:, c0:512],
                            vt[:, ik, 128 * h:128 * (h + 1)],
                            e[:, 512 * ab + c0:512 * (ab + 1)],
                            start=(ik == 0),
                            stop=(ik == nik - 1),
                            skip_group_check=True,
                        )

                pend = None
                for ik in range(nik):
                    for g in dr.get(ik, ()):
                        drain(g)
                    e, c0 = emit_sc(ik)
                    if pend is not None:
                        pump(pump_n)
                        emit_av(*pend)
                    pend = (ik, e, c0)
                pump(pump_n)
                emit_av(*pend)
                # drain the next phase's prerequisites BEFORE the norm: their
                # PE work overlaps the last exp, and their qt/kt casts queue
                # on DVE ahead of the norm ops the next phase doesn't need
                for g in end:
                    drain(g)
                attn_norm(pr, jq, o_ab)

            g_qk01 = gen_qk(0, 1)
            g_qk10 = gen_qk(1, 0)
            g_qk11 = gen_qk(1, 1)
            g_qk20 = gen_qk(2, 0)
            g_qk21 = gen_qk(2, 1)
            g_qk30 = gen_qk(3, 0)
            g_qk31 = gen_qk(3, 1)
            g_v1, g_v2, g_v3 = gen_v(1), gen_v(2), gen_v(3)
            g_p0, g_p1, g_p2 = gen_proj(0), gen_proj(1), gen_proj(2)
            fq.extend([g_qk01, g_qk10, g_v1, g_qk11, g_p0, g_qk20, g_v2,
                       g_qk21, g_p1, g_qk30, g_v3, g_qk31, g_p2])

            run(gen_qk(0, 0))
            run(gen_v(0))
            attn_full(0, 0, {}, end=[g_qk01])
            attn_full(1, 0, {}, end=[g_qk10])
            attn_full(0, 1, {4: [g_v1]}, end=[g_qk11])
            attn_full(1, 1, {}, end=[g_qk20])
            attn_full(0, 2, {8: [g_v2]}, end=[g_qk21])
            attn_full(1, 2, {}, end=[g_qk30])
            attn_full(0, 3, {12: [g_v3]}, end=[g_qk31])
            attn_full(1, 3, {}, pump_n=1)
            drain(g_p2)
            run(gen_proj(3, alternate=True))

    nc.compile()
    return nc


def _get_nc():
    global _CACHED_NC
    if _CACHED_NC is None:
        _CACHED_NC = _build()
    return _CACHED_NC


def _diag_masks() -> np.ndarray:
    return np.ascontiguousarray(np.tril(np.ones((128, 128), dtype=np.float32)).T)


def _in_maps(x, W_qkv, W_proj):
    masks = _diag_masks().astype(ml_dtypes.bfloat16)
    maps = []
    for c in range(N_CORES):
        b, g = divmod(c, 4)
        xT = np.ascontiguousarray(x[b].T.astype(ml_dtypes.bfloat16))
        wq = W_qkv[:, GC * g:GC * (g + 1)]
        wk = W_qkv[:, E + GC * g:E + GC * (g + 1)]
        wv = W_qkv[:, 2 * E + GC * g:2 * E + GC * (g + 1)]
        w = np.ascontiguousarray(
            np.concatenate([wq, wk, wv], axis=1).astype(ml_dtypes.bfloat16))
        wp = np.ascontiguousarray(
            W_proj[GC * g:GC * (g + 1), :].astype(ml_dtypes.bfloat16))
        maps.append({"xT": xT, "w": w, "wp": wp, "mask": masks})
    return maps


def _run(x, W_qkv, W_proj, trace=False, **spmd_kwargs):
    nc = _get_nc()
    res = run_bass_kernel_spmd(nc, _in_maps(x, W_qkv, W_proj),
                               list(range(N_CORES)), trace=trace, **spmd_kwargs)
    out = np.zeros((B, S, E), dtype=np.float32)
    for c in range(N_CORES):
        out[c // 4] += res.results[c]["y"]
    return out, res


def kernel(x, attention_mask, W_qkv, W_proj):
    x = np.asarray(x, dtype=np.float32)
    W_qkv = np.asarray(W_qkv, dtype=np.float32)
    W_proj = np.asarray(W_proj, dtype=np.float32)
    out, _ = _run(x, W_qkv, W_proj, trace=False)
    return out

